# revision 38
# baseline (speedup 1.0000x reference)
"""NSA block (compressed + selected + sliding attention, squared-relu MLP)
on 8 Trainium2 NeuronCores.

Sharding: tensor-parallel over the H=8 attention heads (one head per core);
Wq/Wk/Wv column-sharded, Wo row-sharded with an on-device AllReduce of the
attention output partials; the MLP is tensor-parallel (W1 column- / W2
row-sharded) and its partial sums are reduced on the host while unsharding
(each core returns x2/8 + its MLP partial; the host sums the 8 returns).

Everything is computed in fp32.  Layout strategy: activations that feed
matmul contractions are kept "transposed" (feature dim on partitions,
sequence on the free axis) so that attention scores, attention*V, and all
projections are plain TensorEngine matmuls with no runtime gathers: the
data-dependent top-8 block selection is realized as an additive score mask
built from a per-row 8th-largest threshold (DVE max8) and applied with an
accumulating "expand" matmul.  Softmaxes skip the max-subtraction (scores
are O(7), verified against the reference) so exp sums can be taken with a
ones-column matmul; gate/denominator rows live at partition 0/32/64 because
the BIR verifier requires 32-aligned partition bases.
"""

import sys

sys.path.insert(0, "/opt/trn_rl_repo")

import numpy as np

import concourse.bass as bass
import concourse.mybir as mybir
from concourse.tile import TileContext, add_dep_helper

F32 = mybir.dt.float32
AF = mybir.ActivationFunctionType
ALU = mybir.AluOpType

B, N, DIM, H, D = 1, 1024, 512, 8, 64
BC, BS, K, W = 16, 16, 8, 256
C = N // BC
SCALE = D ** -0.5
EPS = 1e-6
BIG = 1e30
NCORES = 8
NT = N // 128          # 8 query/key tiles of 128
DC = DIM // 128        # 4 dim chunks
HALF = D // 2


def _split_ctrl_waits(nc, cap=1, compute_cap=1):
    """This walrus build's CTRL encoding can't take more than ~1-2 sem waits
    per instruction; hoist excess waits onto chains of preceding Drain clones
    on the same engine."""
    for f in nc.m.functions:
        for bb in f.blocks:
            new_insts = []
            for inst in bb.instructions:
                w = list(inst.sync_info.on_wait) if (inst.sync_info and inst.sync_info.on_wait) else []
                lim = cap if isinstance(inst, (mybir.InstDrain, mybir.InstNoOp)) else compute_cap
                if len(w) > lim and inst.engine is not None:
                    keep_n = max(1, lim)
                    hoist, keep = w[:-keep_n], w[-keep_n:]
                    chunks = [hoist[i:i + max(1, cap)] for i in range(0, len(hoist), max(1, cap))]
                    for ci, chunk in enumerate(chunks):
                        clone = mybir.InstDrain(
                            name=f"{inst.name}_w{ci}",
                            ins=[],
                            outs=[],
                        )
                        clone.engine = inst.engine
                        clone.sync_info = mybir.SyncInfo(on_wait=chunk, on_update=[])
                        new_insts.append(clone)
                    inst.sync_info.on_wait = keep
                new_insts.append(inst)
            bb.instructions[:] = new_insts


def build_program():
    nc = bass.Bass()

    def inp(name, shape):
        return nc.declare_dram_parameter(name, list(shape), F32, isOutput=False)

    # activations
    x_in = inp("x", (N, DIM))
    x0_in = inp("x0", (N, DIM))
    lam_in = inp("lam", (1, 2))
    # per-head weight slices (SCALE folded into wq/wqp on host)
    wq_in = inp("wq", (DIM, D))
    wqp_in = inp("wqp", (DIM, D))
    wk_in = inp("wk", (DIM, D))
    wkp_in = inp("wkp", (DIM, D))
    wv_in = inp("wv", (DIM, D))
    wg_in = inp("wg", (DIM, 96))        # gate b in column 32*b, rest zero
    wo_in = inp("wo", (D, DIM))
    wck_in = inp("wck", (BC * D, D))
    wcv_in = inp("wcv", (BC * D, D))
    ckb_in = inp("ckb", (D, 1))
    cvb_in = inp("cvb", (D, 1))
    kmem_in = inp("kmem", (D, 1))
    vmem_in = inp("vmem", (1, D))
    w1_in = inp("w1", (DIM, 4 * DIM // NCORES))
    w2_in = inp("w2", (4 * DIM // NCORES, DIM))
    # constants
    cosd_in = inp("cosd", (D, N))
    sind_in = inp("sind", (D, N))
    cmn_in = inp("cmask_nat", (N, C + 1))   # mem slot LAST (col C)
    cmt_in = inp("cmask_t", (C + 1, N))
    tri_in = inp("tri_t", (128, 128))
    anti_in = inp("anti_t", (128, 128))
    mown_in = inp("mown_t", (128, 128))
    ownnat_in = inp("ownnat", (128, 120))   # own-block one-hot, sliced per tile
    exp_in = inp("expand", (D, NT, 128))
    rsel_in = inp("rowsel", (3, 128, D))    # rowsel[b, 32b, :] = 1
    ident_in = inp("ident", (128, 128))
    ones65_in = inp("ones65", (C + 1, 1))
    oc32_in = inp("oc32", (128, 33))        # col 32 = ones
    oc64_in = inp("oc64", (128, 65))        # col 64 = ones

    y_out = nc.declare_dram_parameter("y", [N, DIM], F32, isOutput=True)

    ar_in = nc.dram_tensor("ar_in", [N, DIM], F32)
    ar_out = nc.dram_tensor("ar_out", [N, DIM], F32, addr_space="Shared")

    with TileContext(nc) as tc:
        with tc.tile_pool(name="singles", bufs=1) as singles, \
             tc.tile_pool(name="acts", bufs=2) as acts, \
             tc.tile_pool(name="keep", bufs=1) as keep, \
             tc.tile_pool(name="xl_pool", bufs=NT) as xl_pool, \
             tc.tile_pool(name="x2_pool", bufs=NT) as x2_pool, \
             tc.tile_pool(name="tmp", bufs=2) as tmp, \
             tc.tile_pool(name="small", bufs=4) as small, \
             tc.tile_pool(name="ef", bufs=NT + 1) as ef_pool, \
             tc.tile_pool(name="es", bufs=4) as es_pool, \
             tc.tile_pool(name="ps_tr", bufs=3, space="PSUM") as ps_tr, \
             tc.tile_pool(name="ps_av", bufs=2, space="PSUM") as ps_av, \
             tc.tile_pool(name="ps_dn", bufs=1, space="PSUM") as ps_dn, \
             tc.tile_pool(name="ps_big", bufs=2, space="PSUM") as ps_big:

            # ---------------- constants to SBUF ----------------
            _load_n = [0]

            def load(pool, src, shape=None):
                _load_n[0] += 1
                t = pool.tile(list(shape if shape is not None else src.shape), F32,
                              tag=f"const{_load_n[0]}")
                nc.sync.dma_start(t[:], src[:] if shape is None else src)
                return t

            wq = load(singles, wq_in.rearrange("(c p) d -> p c d", p=128))
            wqp = load(singles, wqp_in.rearrange("(c p) d -> p c d", p=128))
            wk = load(singles, wk_in.rearrange("(c p) d -> p c d", p=128))
            wkp = load(singles, wkp_in.rearrange("(c p) d -> p c d", p=128))
            wv = load(singles, wv_in.rearrange("(c p) d -> p c d", p=128))
            wg = load(singles, wg_in.rearrange("(c p) g -> p c g", p=128))
            wo = load(singles, wo_in)
            wck = load(singles, wck_in.rearrange("(b p) e -> p b e", p=D))
            wcv = load(singles, wcv_in.rearrange("(b p) e -> p b e", p=D))
            ckb = load(singles, ckb_in)
            cvb = load(singles, cvb_in)
            w1 = load(singles, w1_in.rearrange("(c p) h -> p c h", p=128))
            w2 = load(singles, w2_in.rearrange("(c p) m -> p c m", p=128))
            cosd = load(singles, cosd_in)
            sind = load(singles, sind_in)
            cmn = load(singles, cmn_in.rearrange("(t p) j -> p t j", p=128))
            cmt = load(singles, cmt_in)
            tri = load(singles, tri_in)
            anti = load(singles, anti_in)
            mown = load(singles, mown_in)
            ownnat = load(singles, ownnat_in)
            expand = load(singles, exp_in)          # [64 part, NT, 128]
            rowsel = load(singles, rsel_in.rearrange("b p d -> p b d"))
            ident = load(singles, ident_in)
            ones65 = load(singles, ones65_in)
            oc32 = load(singles, oc32_in)
            oc64 = load(singles, oc64_in)
            lam_bc = singles.tile([128, 2], F32)
            nc.gpsimd.dma_start(lam_bc[:], lam_in[:].to_broadcast((128, 2)))

            # ---------------- stage 1: x = lam0*x + lam1*x0, rmsnorm, transpose ----------------
            xnT = keep.tile([128, DC, N], F32, tag="xnT")
            xl_tiles = []
            for t in range(NT):
                xt = acts.tile([128, DIM], F32, tag="xload")
                x0t = acts.tile([128, DIM], F32, tag="x0load")
                nc.sync.dma_start(xt[:], x_in[t * 128:(t + 1) * 128, :])
                nc.sync.dma_start(x0t[:], x0_in[t * 128:(t + 1) * 128, :])
                xl = xl_pool.tile([128, DIM], F32, tag="xl")
                nc.vector.tensor_scalar_mul(xl[:], xt[:], lam_bc[:, 0:1])
                t0 = tmp.tile([128, DIM], F32, tag="t0")
                nc.vector.tensor_scalar_mul(t0[:], x0t[:], lam_bc[:, 1:2])
                nc.vector.tensor_add(xl[:], xl[:], t0[:])
                xl_tiles.append(xl)

                sq = tmp.tile([128, DIM], F32, tag="sq")
                ssq = small.tile([128, 1], F32, tag="ssq")
                nc.scalar.activation(sq[:], xl[:], AF.Square, accum_out=ssq[:])
                rn = small.tile([128, 1], F32, tag="rn")
                nc.vector.tensor_scalar(rn[:], ssq[:], 1.0 / DIM, EPS, op0=ALU.mult, op1=ALU.add)
                nc.vector.reciprocal(rn[:], rn[:])
                nc.scalar.activation(rn[:], rn[:], AF.Sqrt)
                xn = acts.tile([128, DIM], F32, tag="xn")
                nc.vector.tensor_scalar_mul(xn[:], xl[:], rn[:, 0:1])
                for c in range(DC):
                    ptr = ps_tr.tile([128, 128], F32, tag="tr")
                    nc.tensor.transpose(ptr[:], xn[:, c * 128:(c + 1) * 128], ident[:])
                    nc.vector.tensor_copy(xnT[:, c, t * 128:(t + 1) * 128], ptr[:])

            # ---------------- stage 3: QKV + gates (T layout) + rope ----------------
            qT = keep.tile([D, N], F32, tag="qT")
            kT = keep.tile([D, N], F32, tag="kT")
            vT = keep.tile([D, N], F32, tag="vT")
            gsig = keep.tile([96, N], F32, tag="gsig")   # gates at rows 0/32/64
            for j in range(2):
                sl = slice(j * 512, (j + 1) * 512)

                def proj(wmat):
                    ps = ps_big.tile([D, 512], F32, tag="big")
                    for c in range(DC):
                        nc.tensor.matmul(ps[:], wmat[:, c, :], xnT[:, c, sl],
                                         start=(c == 0), stop=(c == DC - 1))
                    return ps

                for base, wmat, wpmat in ((qT, wq, wqp), (kT, wk, wkp)):
                    ps_a = proj(wmat)
                    ps_b = proj(wpmat)
                    rt = tmp.tile([D, 512], F32, tag="ropet")
                    nc.vector.tensor_mul(base[:, sl], ps_a[:], cosd[:, sl])
                    nc.vector.tensor_mul(rt[:], ps_b[:], sind[:, sl])
                    nc.vector.tensor_add(base[:, sl], base[:, sl], rt[:])
                ps_v = proj(wv)
                nc.vector.tensor_copy(vT[:, sl], ps_v[:])
                psg = ps_big.tile([96, 512], F32, tag="big")
                for c in range(DC):
                    nc.tensor.matmul(psg[:], wg[:, c, :], xnT[:, c, sl],
                                     start=(c == 0), stop=(c == DC - 1))
                nc.scalar.activation(gsig[:, sl], psg[:], AF.Sigmoid)

            # ---------------- stage 4: v_nat ----------------
            v_nat = keep.tile([128, NT, D], F32, tag="v_nat")
            for t in range(NT):
                ptr = ps_tr.tile([128, 128], F32, tag="tr")
                nc.tensor.transpose(ptr[:, :D], vT[:, t * 128:(t + 1) * 128], ident[:D, :D])
                nc.vector.tensor_copy(v_nat[:, t, :], ptr[:, :D])

            # ---------------- stage 5: compressed branch ----------------
            # ck_T/cv_T [64e, 64c] via 16 accumulated b-matmuls on strided kT/vT
            ckf = keep.tile([D, C + 1], F32, tag="ckf")     # cols 0..63 ck, col 64 kmem
            nc.sync.dma_start(ckf[:, C:C + 1], kmem_in[:])
            kT_b = kT[:].rearrange("d (c b) -> d b c", b=BC)
            vT_b = vT[:].rearrange("d (c b) -> d b c", b=BC)
            ps_ck = ps_av.tile([D, C], F32, tag="av")
            for b in range(BC):
                nc.tensor.matmul(ps_ck[:], wck[:, b, :], kT_b[:, b, :],
                                 start=(b == 0), stop=(b == BC - 1))
            nc.vector.tensor_scalar_add(ckf[:, :C], ps_ck[:], ckb[:, 0:1])

            cv_sb = small.tile([D, C], F32, tag="cv")
            ps_cv = ps_av.tile([D, C], F32, tag="av")
            for b in range(BC):
                nc.tensor.matmul(ps_cv[:], wcv[:, b, :], vT_b[:, b, :],
                                 start=(b == 0), stop=(b == BC - 1))
            nc.vector.tensor_scalar_add(cv_sb[:], ps_cv[:], cvb[:, 0:1])
            # cvf_nat [65, 64]: rows 0..63 = cv blocks, row 64 = vmem
            cvf = keep.tile([C + 1, D], F32, tag="cvf")
            nc.sync.dma_start(cvf[C:C + 1, :], vmem_in[:])
            ptr = ps_tr.tile([128, 128], F32, tag="tr")
            nc.tensor.transpose(ptr[:C, :C], cv_sb[:], ident[:D, :D])
            nc.vector.tensor_copy(cvf[0:C, :], ptr[:C, :D])

            # coef_pad rows: 0 = compressed, 32 = fine, 64 = sliding
            coef = keep.tile([128, N], F32, tag="coef")
            nc.vector.memset(coef[:], 0.0)

            # csim in T layout -> E_ct; c_out = cvf^T @ E_ct; denom row 0
            E_ct = keep.tile([C + 1, N], F32, tag="E_ct")
            coT = keep.tile([D, N], F32, tag="coT")
            for j in range(2):
                sl = slice(j * 512, (j + 1) * 512)
                ps = ps_big.tile([C + 1, 512], F32, tag="big")
                nc.tensor.matmul(ps[:], ckf[:], qT[:, sl], start=True, stop=True)
                nc.vector.tensor_add(ps[:], ps[:], cmt[:, sl])
                nc.scalar.activation(E_ct[:, sl], ps[:], AF.Exp)
                ps2 = ps_big.tile([D, 512], F32, tag="big")
                nc.tensor.matmul(ps2[:], cvf[:], E_ct[:, sl], start=True, stop=True)
                nc.vector.tensor_copy(coT[:, sl], ps2[:])
                psd = ps_dn.tile([1, 512], F32, tag="dn")
                nc.tensor.matmul(psd[:], ones65[:], E_ct[:, sl], start=True, stop=True)
                nc.vector.tensor_copy(coef[0:1, sl], psd[:])

            # csim natural per q-tile -> top-8 threshold -> (sel-1) -> transpose
            selm1T = keep.tile([D, NT, 128], F32, tag="selm1T")
            for t in range(NT):
                ps = ps_tr.tile([128, C + 1], F32, tag="tr")
                nc.tensor.matmul(ps[:], qT[:, t * 128:(t + 1) * 128], ckf[:],
                                 start=True, stop=True)
                nc.vector.tensor_add(ps[:], ps[:], cmn[:, t, :])
                ecn = small.tile([128, C + 1], F32, tag="ecn")
                nc.scalar.activation(ecn[:], ps[:], AF.Exp)
                t8 = small.tile([128, 8], F32, tag="t8")
                nc.vector.max(t8[:], ecn[:, 0:C])
                tt = small.tile([128, 1], F32, tag="tt")
                nc.vector.tensor_scalar_max(tt[:], t8[:, 7:8], 1e-38)
                selm1 = small.tile([128, C], F32, tag="selm1")
                nc.vector.tensor_tensor(selm1[:], ecn[:, 0:C],
                                        tt[:].to_broadcast((128, C)), ALU.is_ge)
                # mark the own block as "selected" so the expand matmul does
                # not annihilate its scores on the diagonal tile (off-diagonal
                # expand matmuls never read these rows); causality inside the
                # own block is enforced by the subtractive mown mask.
                nc.vector.tensor_add(selm1[:], selm1[:],
                                     ownnat[:, 56 - 8 * t:120 - 8 * t])
                nc.vector.tensor_scalar_add(selm1[:], selm1[:], -1.0)
                ptr = ps_tr.tile([128, 128], F32, tag="tr")
                nc.tensor.transpose(ptr[:C, :], selm1[:], ident[:])
                nc.vector.tensor_copy(selm1T[:, t, :], ptr[:C, :])

            # ---------------- stage 6: fine + sliding branches, gating, Wo ----------------
            oT = keep.tile([D, N], F32, tag="oT")
            foT = keep.tile([D, N], F32, tag="foT")
            soT = keep.tile([D, N], F32, tag="soT")
            ar_stores = []
            for t in range(NT):
                qsl = slice(t * 128, (t + 1) * 128)
                # fine: all past tiles + own
                ef_tiles = []
                for jt in range(t + 1):
                    ps = ps_tr.tile([128, 128], F32, tag="tr")
                    nc.tensor.matmul(ps[:], kT[:, jt * 128:(jt + 1) * 128], qT[:, qsl],
                                     start=True, stop=False)
                    nc.tensor.matmul(ps[:], expand[:, jt, :], selm1T[:, t, :],
                                     start=False, stop=True)
                    if jt == t:
                        nc.vector.tensor_add(ps[:], ps[:], mown[:])
                    ef = ef_pool.tile([128, 128], F32, tag="ef")
                    nc.scalar.activation(ef[:], ps[:], AF.Exp)
                    ef_tiles.append(ef)
                ps_f = ps_av.tile([D, 128], F32, tag="av")
                for jt in range(t + 1):
                    nc.tensor.matmul(ps_f[:], v_nat[:, jt, :], ef_tiles[jt][:],
                                     start=(jt == 0), stop=(jt == t))
                nc.vector.tensor_copy(foT[:, qsl], ps_f[:])
                psd = ps_dn.tile([33, 128], F32, tag="dn")
                for jt in range(t + 1):
                    nc.tensor.matmul(psd[:], oc32[:, :], ef_tiles[jt][:],
                                     start=(jt == 0), stop=(jt == t))
                nc.vector.tensor_copy(coef[32:33, qsl], psd[32:33, :])

                # sliding: tiles t-2, t-1, t
                es_tiles = {}
                for jt in range(max(0, t - 2), t + 1):
                    ps = ps_tr.tile([128, 128], F32, tag="tr")
                    nc.tensor.matmul(ps[:], kT[:, jt * 128:(jt + 1) * 128], qT[:, qsl],
                                     start=True, stop=True)
                    if jt == t:
                        nc.vector.tensor_add(ps[:], ps[:], tri[:])
                    elif jt == t - 2:
                        nc.vector.tensor_add(ps[:], ps[:], anti[:])
                    es = es_pool.tile([128, 128], F32, tag="es")
                    nc.scalar.activation(es[:], ps[:], AF.Exp)
                    es_tiles[jt] = es
                ps_s = ps_av.tile([D, 128], F32, tag="av")
                jts = sorted(es_tiles.keys())
                for jt in jts:
                    nc.tensor.matmul(ps_s[:], v_nat[:, jt, :], es_tiles[jt][:],
                                     start=(jt == jts[0]), stop=(jt == jts[-1]))
                nc.vector.tensor_copy(soT[:, qsl], ps_s[:])
                psd2 = ps_dn.tile([65, 128], F32, tag="dn")
                for jt in jts:
                    nc.tensor.matmul(psd2[:], oc64[:, :], es_tiles[jt][:],
                                     start=(jt == jts[0]), stop=(jt == jts[-1]))
                nc.vector.tensor_copy(coef[64:65, qsl], psd2[64:65, :])

            # coefs = sigmoid(gate) / denom  (rows 0/32/64)
            for r in (0, 32, 64):
                nc.vector.reciprocal(coef[r:r + 1, :], coef[r:r + 1, :])
                nc.vector.tensor_mul(coef[r:r + 1, :], coef[r:r + 1, :], gsig[r:r + 1, :])

            # combine branches + Wo partial
            for t in range(NT):
                qsl = slice(t * 128, (t + 1) * 128)
                bc = []
                for bidx in range(3):
                    pb = ps_tr.tile([D, 128], F32, tag="tr")
                    nc.tensor.matmul(pb[:], rowsel[:, bidx, :], coef[:, qsl],
                                     start=True, stop=True)
                    bc.append(pb)
                ot = oT[:, qsl]
                nc.vector.tensor_mul(ot, coT[:, qsl], bc[0][:])
                tt2 = tmp.tile([D, 128], F32, tag="comb")
                nc.vector.tensor_mul(tt2[:], foT[:, qsl], bc[1][:])
                nc.vector.tensor_add(ot, ot, tt2[:])
                nc.vector.tensor_mul(tt2[:], soT[:, qsl], bc[2][:])
                nc.vector.tensor_add(ot, ot, tt2[:])

                ps_wo = ps_big.tile([128, DIM], F32, tag="big")
                nc.tensor.matmul(ps_wo[:], oT[:, qsl], wo[:], start=True, stop=True)
                attn = tmp.tile([128, DIM], F32, tag="attn")
                nc.vector.tensor_copy(attn[:], ps_wo[:])
                st = nc.sync.dma_start(ar_in[t * 128:(t + 1) * 128, :], attn[:])
                ar_stores.append(st.ins if hasattr(st, "ins") else st)

            # ---------------- stage 7: AllReduce (split in halves for overlap) ----------------
            cc_insts = []
            for half in range(2):
                rsl = slice(half * 512, (half + 1) * 512)
                cc = nc.gpsimd.collective_compute(
                    "AllReduce",
                    ALU.add,
                    ins=[ar_in[rsl, :]],
                    outs=[ar_out[rsl, :]],
                    replica_groups=[list(range(NCORES))],
                )
                cc_inst = cc.ins if hasattr(cc, "ins") else cc
                for st in ar_stores[half * 4:(half + 1) * 4]:
                    add_dep_helper(cc_inst, st, reason="allreduce waits on attn stores")
                cc_insts.append(cc_inst)

            mnT = keep.tile([128, DC, N], F32, tag="xnT")  # reuse xnT slot (dead by now)
            x2_tiles = []
            for t in range(NT):
                art = acts.tile([128, DIM], F32, tag="arload")
                ld = nc.sync.dma_start(art[:], ar_out[t * 128:(t + 1) * 128, :])
                add_dep_helper(ld.ins if hasattr(ld, "ins") else ld, cc_insts[t // 4],
                               reason="ar_out load waits on allreduce")
                x2 = x2_pool.tile([128, DIM], F32, tag="x2")
                nc.vector.tensor_add(x2[:], xl_tiles[t][:], art[:])
                x2_tiles.append(x2)
                sq = tmp.tile([128, DIM], F32, tag="sq")
                ssq = small.tile([128, 1], F32, tag="ssq")
                nc.scalar.activation(sq[:], x2[:], AF.Square, accum_out=ssq[:])
                rn = small.tile([128, 1], F32, tag="rn")
                nc.vector.tensor_scalar(rn[:], ssq[:], 1.0 / DIM, EPS, op0=ALU.mult, op1=ALU.add)
                nc.vector.reciprocal(rn[:], rn[:])
                nc.scalar.activation(rn[:], rn[:], AF.Sqrt)
                mn = acts.tile([128, DIM], F32, tag="mn")
                nc.vector.tensor_scalar_mul(mn[:], x2[:], rn[:, 0:1])
                for c in range(DC):
                    ptr = ps_tr.tile([128, 128], F32, tag="tr")
                    nc.tensor.transpose(ptr[:], mn[:, c * 128:(c + 1) * 128], ident[:])
                    nc.vector.tensor_copy(mnT[:, c, t * 128:(t + 1) * 128], ptr[:])

            # ---------------- stage 8: MLP (column/row sharded) ----------------
            HS = 4 * DIM // NCORES  # 256 hidden per core
            hsq = keep.tile([128, HS // 128, N], F32, tag="hsq")
            for half in range(HS // 128):
                for j in range(2):
                    sl = slice(j * 512, (j + 1) * 512)
                    ps = ps_big.tile([128, 512], F32, tag="big")
                    for c in range(DC):
                        nc.tensor.matmul(ps[:], w1[:, c, half * 128:(half + 1) * 128],
                                         mnT[:, c, sl], start=(c == 0), stop=(c == DC - 1))
                    hr = tmp.tile([128, 512], F32, tag="hr")
                    nc.scalar.activation(hr[:], ps[:], AF.Relu)
                    nc.vector.tensor_mul(hsq[:, half, sl], hr[:], hr[:])
            for t in range(NT):
                ps = ps_big.tile([128, DIM], F32, tag="big")
                for half in range(HS // 128):
                    nc.tensor.matmul(ps[:], hsq[:, half, t * 128:(t + 1) * 128],
                                     w2[:, half, :], start=(half == 0), stop=(half == HS // 128 - 1))
                yt = tmp.tile([128, DIM], F32, tag="yt")
                nc.vector.tensor_scalar(yt[:], x2_tiles[t][:], 1.0 / NCORES, None, op0=ALU.mult)
                nc.vector.tensor_add(yt[:], yt[:], ps[:])
                nc.sync.dma_start(y_out[t * 128:(t + 1) * 128, :], yt[:])

    _split_ctrl_waits(nc, cap=1)
    return nc


_NC_CACHE = None
TRACE = False        # set True (e.g. from test.py) to capture an NTFF profile
LAST_RESULT = None   # BassKernelResults of the most recent run


def _get_program():
    global _NC_CACHE
    if _NC_CACHE is None:
        _NC_CACHE = build_program()
    return _NC_CACHE


def _host_constants():
    half = HALF
    # match the reference's fp32 angle arithmetic exactly
    inv = (1.0 / (10000.0 ** (np.arange(half, dtype=np.float32) / np.float32(half)))).astype(np.float32)
    ang = (np.arange(N, dtype=np.float32)[:, None] * inv[None, :]).astype(np.float32)
    cos_t, sin_t = np.cos(ang).T.astype(np.float32), np.sin(ang).T.astype(np.float32)
    cosd = np.concatenate([cos_t, cos_t], axis=0).astype(np.float32)
    sind = np.concatenate([sin_t, sin_t], axis=0).astype(np.float32)

    i_idx = np.arange(N)
    past = ((np.arange(C) + 1) * BC - 1)[None, :] < i_idx[:, None]   # [N, C]
    cmn = np.zeros((N, C + 1), np.float32)      # mem slot last (col C) = 0
    cmn[:, :C][~past] = -BIG
    cmt = np.ascontiguousarray(cmn.T)

    jl = np.arange(128)[:, None]
    ql = np.arange(128)[None, :]
    tri = np.where(jl <= ql, 0.0, -BIG).astype(np.float32)
    anti = np.where(jl > ql, 0.0, -BIG).astype(np.float32)
    mown = np.where((jl // BS == ql // BS) & (jl > ql), -BIG, 0.0).astype(np.float32)

    ownnat = np.zeros((128, 120), np.float32)
    for qloc in range(128):
        ownnat[qloc, 56 + qloc // BS] = 1.0

    expand = np.zeros((D, NT, 128), np.float32)
    for jt in range(NT):
        for j in range(128):
            expand[8 * jt + j // BS, jt, j] = BIG

    rowsel = np.zeros((3, 128, D), np.float32)
    for b in range(3):
        rowsel[b, 32 * b, :] = 1.0

    ident = np.eye(128, dtype=np.float32)
    ones65 = np.ones((C + 1, 1), np.float32)
    oc32 = np.zeros((128, 33), np.float32); oc32[:, 32] = 1.0
    oc64 = np.zeros((128, 65), np.float32); oc64[:, 64] = 1.0
    return dict(cosd=cosd, sind=sind, cmask_nat=cmn, cmask_t=cmt, tri_t=tri,
                anti_t=anti, mown_t=mown, ownnat=ownnat, expand=expand,
                rowsel=rowsel, ident=ident, ones65=ones65, oc32=oc32, oc64=oc64)


def kernel(x, ve, x0, lambdas, Wq, Wk, Wv, Wo, Wg, k_pos, v_pos, Wck, bck, Wcv,
           bcv, k_mem, v_mem, W1, W2):
    from concourse.bass_utils import run_bass_kernel_spmd

    nc = _get_program()
    consts = _host_constants()

    x = np.asarray(x, np.float32).reshape(N, DIM)
    x0 = np.asarray(x0, np.float32).reshape(N, DIM)
    lam = np.asarray(lambdas, np.float32).reshape(1, 2)
    Wq, Wk, Wv = (np.asarray(a, np.float32) for a in (Wq, Wk, Wv))
    Wo, Wg = np.asarray(Wo, np.float32), np.asarray(Wg, np.float32)
    Wck, Wcv = np.asarray(Wck, np.float32), np.asarray(Wcv, np.float32)
    bck, bcv = np.asarray(bck, np.float32), np.asarray(bcv, np.float32)
    k_pos, v_pos = np.asarray(k_pos, np.float32), np.asarray(v_pos, np.float32)
    k_mem, v_mem = np.asarray(k_mem, np.float32), np.asarray(v_mem, np.float32)
    W1, W2 = np.asarray(W1, np.float32), np.asarray(W2, np.float32)

    def rot(wm):
        # (P q)[d] = -q[d+32] (d<32) ; q[d-32] (d>=32)
        return np.concatenate([-wm[:, HALF:D], wm[:, :HALF]], axis=1)

    HS = 4 * DIM // NCORES
    in_maps = []
    for h in range(NCORES):
        wq_h = Wq[:, h * D:(h + 1) * D]
        wk_h = Wk[:, h * D:(h + 1) * D]
        ckb = (bck + np.einsum("bd,bde->e", k_pos[h],
                               Wck.reshape(BC, D, D))).astype(np.float32)
        cvb = (bcv + np.einsum("bd,bde->e", v_pos[h],
                               Wcv.reshape(BC, D, D))).astype(np.float32)
        wg96 = np.zeros((DIM, 96), np.float32)
        for b in range(3):
            wg96[:, 32 * b] = Wg[:, 3 * h + b]
        im = dict(
            x=x, x0=x0, lam=lam,
            wq=np.ascontiguousarray(wq_h * SCALE),
            wqp=np.ascontiguousarray(rot(wq_h) * SCALE),
            wk=np.ascontiguousarray(wk_h),
            wkp=np.ascontiguousarray(rot(wk_h)),
            wv=np.ascontiguousarray(Wv[:, h * D:(h + 1) * D]),
            wg=wg96,
            wo=np.ascontiguousarray(Wo[h * D:(h + 1) * D, :]),
            wck=Wck, wcv=Wcv,
            ckb=ckb.reshape(D, 1), cvb=cvb.reshape(D, 1),
            kmem=np.ascontiguousarray(k_mem[h].reshape(D, 1)),
            vmem=np.ascontiguousarray(v_mem[h].reshape(1, D)),
            w1=np.ascontiguousarray(W1[:, h * HS:(h + 1) * HS]),
            w2=np.ascontiguousarray(W2[h * HS:(h + 1) * HS, :]),
            **consts,
        )
        in_maps.append({k: np.ascontiguousarray(v, np.float32) if v.dtype != np.float32 else np.ascontiguousarray(v)
                        for k, v in im.items()})

    global LAST_RESULT
    res = run_bass_kernel_spmd(nc, in_maps, list(range(NCORES)), trace=TRACE)
    LAST_RESULT = res
    y = np.zeros((N, DIM), np.float32)
    for r in res.results:
        y += r["y"]
    return y.reshape(B, N, DIM)
